# revision 39
# baseline (speedup 1.0000x reference)
"""Trainium2 Bass kernel for a 6-layer bigram language model (dense transformer).

Data-parallel over batch: B=64 -> 8 sequences per NeuronCore, 8 cores.
Activations kept D-major (xT [D, tokens]); all matmuls in float32r
(full PE rate, ~1e-3 max rel err). Single ACT table set (exp/ln/relu/
square/copy). Returns (logits [64,256,65] f32, loss scalar f32).
"""
import ml_dtypes
import numpy as np

import concourse.bass as bass
import concourse.mybir as mybir
import concourse.tile as tile
from concourse import bacc, bass_utils
from concourse.alu_op_type import AluOpType

F32 = mybir.dt.float32
F32R = mybir.dt.float32r
BF16 = mybir.dt.bfloat16
ACT = mybir.ActivationFunctionType

L, H, HS, D, T, V, B = 6, 6, 64, 384, 256, 65, 64
EPS = 1e-5
NCORES = 8
BL = B // NCORES            # sequences per core = 8
NTOK = BL * T               # 2048 tokens per core
NG = 4                      # token groups per core
TG = NTOK // NG             # 512 tokens per group (2 sequences)
DC = D // 128               # 3 d-chunks
FF = 4 * D                  # 1536
FC = FF // 128              # 12 ffn chunks
NEG = -1.0e30

_cache = {}


def _build_program(flags):
    nc = bacc.Bacc("TRN2", target_bir_lowering=False, debug=False)

    # ---- DRAM tensors ----
    IOH = nc.dram_tensor("IOH", [128, NTOK], F32R, kind="ExternalInput")
    EMB = nc.dram_tensor("EMB", [128, D], F32R, kind="ExternalInput")
    POS = nc.dram_tensor("POS", [DC, 128, T], F32, kind="ExternalInput")
    WQ = nc.dram_tensor("WQ", [L, D, D], F32R, kind="ExternalInput")
    WK = nc.dram_tensor("WK", [L, D, D], F32R, kind="ExternalInput")
    WV = nc.dram_tensor("WV", [L, D, D], F32R, kind="ExternalInput")
    WP = nc.dram_tensor("WP", [L, D, D], F32R, kind="ExternalInput")
    W1 = nc.dram_tensor("W1", [L, D, FF], F32R, kind="ExternalInput")
    W2 = nc.dram_tensor("W2", [L, FF, D], F32R, kind="ExternalInput")
    WH = nc.dram_tensor("WH", [D, V + 1], F32R, kind="ExternalInput")
    MSK = nc.dram_tensor("MSK", [128, 128], BF16, kind="ExternalInput")
    MSKF = nc.dram_tensor("MSKF", [128, 128], BF16, kind="ExternalInput")
    IDN = nc.dram_tensor("IDN", [128, 128], BF16, kind="ExternalInput")
    ON128 = nc.dram_tensor("ON128", [128, 1], F32R, kind="ExternalInput")
    ONB = nc.dram_tensor("ONB", [128, 1], BF16, kind="ExternalInput")
    ONK1 = nc.dram_tensor("ONK1", [1, 128], F32R, kind="ExternalInput")
    SELLO = nc.dram_tensor("SELLO", [1, 128], F32R, kind="ExternalInput")
    SELHI = nc.dram_tensor("SELHI", [1, 128], F32R, kind="ExternalInput")
    OHT = nc.dram_tensor("OHT", [NTOK, V], BF16, kind="ExternalInput")
    # per-layer per-partition bias vectors (may be unused if all-zero)
    BPR = nc.dram_tensor("BPR", [L, DC, 128], F32, kind="ExternalInput")
    B1 = nc.dram_tensor("B1", [L, FC, 128], F32, kind="ExternalInput")
    B2 = nc.dram_tensor("B2", [L, DC, 128], F32, kind="ExternalInput")
    # LN affine (used only when nontrivial)
    LG1 = nc.dram_tensor("LG1", [L, DC, 128], F32, kind="ExternalInput")
    LB1 = nc.dram_tensor("LB1", [L, DC, 128], F32, kind="ExternalInput")
    LGF = nc.dram_tensor("LGF", [DC, 128], F32, kind="ExternalInput")
    LBF = nc.dram_tensor("LBF", [DC, 128], F32, kind="ExternalInput")
    BHD = nc.dram_tensor("BHD", [128, V], F32, kind="ExternalInput")

    LOGITS = nc.dram_tensor("LOGITS", [NTOK, V], F32, kind="ExternalOutput")
    LOSS = nc.dram_tensor("LOSSN", [1, 1], F32, kind="ExternalOutput")

    with tile.TileContext(nc) as tc:
        with (
            tc.tile_pool(name="persist", bufs=1) as persist,
            tc.tile_pool(name="wts", bufs=1) as wts,
            tc.tile_pool(name="wts1", bufs=1) as wts1,
            tc.tile_pool(name="acts", bufs=1) as acts,
            tc.tile_pool(name="lnp", bufs=2) as lnp,
            tc.tile_pool(name="sm", bufs=2) as sm,
            tc.tile_pool(name="wat", bufs=2) as wat,
            tc.tile_pool(name="io", bufs=2) as io,
            tc.tile_pool(name="pmm", bufs=3, space="PSUM") as pmm,
            tc.tile_pool(name="pat", bufs=3, space="PSUM") as pat,
            tc.tile_pool(name="pvec", bufs=2, space="PSUM") as pvec,
        ):
            # ---------- constants ----------
            xT = persist.tile([128, DC, NTOK], F32R, tag="xT")
            emb = persist.tile([128, D], F32R, tag="emb")
            nc.sync.dma_start(out=emb, in_=EMB[:, :])
            pos = persist.tile([128, DC, T], F32, tag="pos")
            nc.sync.dma_start(out=pos, in_=POS[:, :, :].rearrange("c p t -> p c t"))
            msk = persist.tile([128, 128], BF16, tag="msk")
            nc.sync.dma_start(out=msk, in_=MSK[:, :])
            mskf = persist.tile([128, 128], BF16, tag="mskf")
            nc.sync.dma_start(out=mskf, in_=MSKF[:, :])
            idn = persist.tile([128, 128], BF16, tag="idn")
            nc.sync.dma_start(out=idn, in_=IDN[:, :])
            on128 = persist.tile([128, 1], F32R, tag="on128")
            nc.sync.dma_start(out=on128, in_=ON128[:, :])
            onb = persist.tile([128, 1], BF16, tag="onb")
            nc.sync.dma_start(out=onb, in_=ONB[:, :])
            onk1 = persist.tile([1, 128], F32R, tag="onk1")
            nc.sync.dma_start(out=onk1, in_=ONK1[:, :])
            sello = persist.tile([1, 128], F32R, tag="sello")
            nc.sync.dma_start(out=sello, in_=SELLO[:, :])
            selhi = persist.tile([1, 128], F32R, tag="selhi")
            nc.sync.dma_start(out=selhi, in_=SELHI[:, :])
            whd = persist.tile([128, DC, V + 1], F32R, tag="whd")
            nc.sync.dma_start(out=whd, in_=WH.rearrange("(c p) v -> p c v", p=128))

            acc = persist.tile([128, 2], F32, tag="acc")
            nc.vector.memset(acc, 0.0)
            ses = persist.tile([128, 16], F32, tag="ses")
            dts = persist.tile([128, 16], F32, tag="dts")
            eps_sb = persist.tile([1, 1], F32, tag="eps")
            nc.vector.memset(eps_sb, EPS)

            bpr_sb = persist.tile([128, L, DC], F32, tag="bpr")
            b2_sb = persist.tile([128, L, DC], F32, tag="b2")
            b1_sb = persist.tile([128, L, FC], F32, tag="b1")
            if flags["bpr"]:
                nc.sync.dma_start(out=bpr_sb, in_=BPR.rearrange("l c p -> p l c"))
            if flags["b2"]:
                nc.sync.dma_start(out=b2_sb, in_=B2.rearrange("l c p -> p l c"))
            if flags["b1"]:
                nc.sync.dma_start(out=b1_sb, in_=B1.rearrange("l c p -> p l c"))
            lg1_sb = persist.tile([128, L, DC], F32, tag="lg1")
            lb1_sb = persist.tile([128, L, DC], F32, tag="lb1")
            lgf_sb = persist.tile([128, DC], F32, tag="lgf")
            lbf_sb = persist.tile([128, DC], F32, tag="lbf")
            if flags["ln1"]:
                nc.sync.dma_start(out=lg1_sb, in_=LG1.rearrange("l c p -> p l c"))
                nc.sync.dma_start(out=lb1_sb, in_=LB1.rearrange("l c p -> p l c"))
            if flags["lnf"]:
                nc.sync.dma_start(out=lgf_sb, in_=LGF.rearrange("c p -> p c"))
                nc.sync.dma_start(out=lbf_sb, in_=LBF.rearrange("c p -> p c"))
            bhd_sb = persist.tile([128, V], F32, tag="bhd")
            if flags["bhd"]:
                nc.sync.dma_start(out=bhd_sb, in_=BHD[:, :])

            # ---------- embedding ----------
            for g in range(NG):
                gsl = slice(g * TG, (g + 1) * TG)
                i_sb = io.tile([128, TG], F32R, tag="ioh")
                nc.sync.dma_start(out=i_sb, in_=IOH[:, gsl])
                for c in range(DC):
                    pe = pmm.tile([128, TG], F32, tag="mm512")
                    nc.tensor.matmul(
                        pe[:], emb[:, c * 128:(c + 1) * 128], i_sb[:],
                        start=True, stop=True,
                    )
                    for half in range(2):
                        hsl = slice(half * T, (half + 1) * T)
                        nc.vector.tensor_add(
                            out=xT[:, c, g * TG + half * T: g * TG + (half + 1) * T],
                            in0=pe[:, hsl], in1=pos[:, c, :],
                        )

            # ---------- layer norm helper ----------
            def layer_norm(src_view, dst, affine):
                """src_view: [128, DC, TG] f32r AP; dst [128, DC, TG] f32r tile."""
                sq = lnp.tile([128, DC, TG], F32R, tag="sq")
                for c in range(DC):
                    nc.scalar.activation(sq[:, c, :], src_view[:, c, :], ACT.Square)
                ps1 = pvec.tile([1, TG], F32, tag="vec")
                ps2 = pvec.tile([1, TG], F32, tag="vec")
                for c in range(DC):
                    nc.tensor.matmul(ps1[:], on128[:], src_view[:, c, :],
                                     start=(c == 0), stop=(c == DC - 1))
                    nc.tensor.matmul(ps2[:], on128[:], sq[:, c, :],
                                     start=(c == 0), stop=(c == DC - 1))
                m = sm.tile([1, TG], F32, tag="stat_m")
                nc.scalar.activation(m[:], ps1[:], ACT.Copy, scale=1.0 / D)
                mm = sm.tile([1, TG], F32, tag="stat")
                nc.vector.tensor_mul(out=mm[:], in0=m[:], in1=m[:])
                var = sm.tile([1, TG], F32, tag="stat")
                nc.vector.scalar_tensor_tensor(
                    out=var[:], in0=ps2[:], scalar=1.0 / D, in1=mm[:],
                    op0=AluOpType.mult, op1=AluOpType.subtract,
                )
                lnv = sm.tile([1, TG], F32, tag="stat")
                nc.scalar.activation(lnv[:], var[:], ACT.Ln, bias=eps_sb[:])
                a_r = sm.tile([1, TG], F32R, tag="statr")
                nc.scalar.activation(a_r[:], lnv[:], ACT.Exp, scale=-0.5)
                ma_r = sm.tile([1, TG], F32R, tag="statr")
                nc.vector.tensor_mul(out=ma_r[:], in0=m[:], in1=a_r[:])
                pa = pat.tile([128, TG], F32, tag="at")
                nc.tensor.matmul(pa[:], onk1[:], a_r[:], start=True, stop=True)
                pma = pat.tile([128, TG], F32, tag="at")
                nc.tensor.matmul(pma[:], onk1[:], ma_r[:], start=True, stop=True)
                for c in range(DC):
                    u = io.tile([128, TG], F32, tag="u")
                    nc.vector.tensor_mul(out=u[:], in0=src_view[:, c, :], in1=pa[:])
                    if affine is None:
                        nc.vector.tensor_sub(out=dst[:, c, :], in0=u[:], in1=pma[:])
                    else:
                        nc.vector.tensor_sub(out=u[:], in0=u[:], in1=pma[:])
                        gs, bs = affine
                        nc.vector.tensor_scalar(
                            out=dst[:, c, :], in0=u[:],
                            scalar1=gs(c), scalar2=bs(c),
                            op0=AluOpType.mult, op1=AluOpType.add,
                        )

            # ---------- transformer layers ----------
            pending_yT0 = None
            for l in range(L):
                wq_sb = wts.tile([128, DC, D], F32R, tag="wq")
                nc.sync.dma_start(out=wq_sb, in_=WQ[l].rearrange("(c p) e -> p c e", p=128))
                wk_sb = wts.tile([128, DC, D], F32R, tag="wk")
                nc.sync.dma_start(out=wk_sb, in_=WK[l].rearrange("(c p) e -> p c e", p=128))
                wv_sb = wts.tile([128, DC, D], F32R, tag="wv")
                nc.sync.dma_start(out=wv_sb, in_=WV[l].rearrange("(c p) e -> p c e", p=128))
                wp_sb = wts.tile([128, DC, D], F32R, tag="wp")
                nc.sync.dma_start(out=wp_sb, in_=WP[l].rearrange("(c p) e -> p c e", p=128))
                w1_sb = wts1.tile([128, DC, FF], F32R, tag="w1")
                nc.sync.dma_start(out=w1_sb, in_=W1[l].rearrange("(c p) e -> p c e", p=128))
                w2_sb = wts1.tile([128, FC, D], F32R, tag="w2")
                nc.sync.dma_start(out=w2_sb, in_=W2[l].rearrange("(c p) e -> p c e", p=128))

                if flags["ln1"]:
                    aff1 = (
                        lambda c, l=l: lg1_sb[:, l, c:c + 1],
                        lambda c, l=l: lb1_sb[:, l, c:c + 1],
                    )
                else:
                    aff1 = None

                def qkv_phase(g, yT):
                    gsl = slice(g * TG, (g + 1) * TG)
                    qT = acts.tile([128, DC, TG], F32R, tag="qT", name=f"qT{g}")
                    kT = acts.tile([128, DC, TG], F32R, tag="kT", name=f"kT{g}")
                    for ec in range(DC):
                        pq = pmm.tile([128, TG], F32, tag="mm512")
                        pk = pmm.tile([128, TG], F32, tag="mm512")
                        for kc in range(DC):
                            esl = slice(ec * 128, (ec + 1) * 128)
                            nc.tensor.matmul(pq[:], wq_sb[:, kc, esl], yT[:, kc, :],
                                             start=(kc == 0), stop=(kc == DC - 1))
                            nc.tensor.matmul(pk[:], wk_sb[:, kc, esl], yT[:, kc, :],
                                             start=(kc == 0), stop=(kc == DC - 1))
                        nc.vector.tensor_copy(out=qT[:, ec, :], in_=pq[:])
                        nc.vector.tensor_copy(out=kT[:, ec, :], in_=pk[:])
                    v_sb = acts.tile([128, 4, D], F32R, tag="v", name=f"v{g}")
                    for j in range(4):
                        pv = pmm.tile([128, TG], F32, tag="mm512")
                        tsl = slice(j * 128, (j + 1) * 128)
                        for kc in range(DC):
                            nc.tensor.matmul(pv[:, :D], yT[:, kc, tsl], wv_sb[:, kc, :],
                                             start=(kc == 0), stop=(kc == DC - 1))
                        nc.scalar.activation(v_sb[:, j, :], pv[:, :D], ACT.Copy)
                    return qT, kT, v_sb

                def attn_phase(g, qT, kT, v_sb):
                    oT = acts.tile([128, DC, TG], F32R, tag="oT", name=f"oT{g}")
                    iters = [(ib, j) for ib in range(2) for j in range(DC)]

                    def stage_s(i):
                        ib, j = iters[i]
                        t0 = ib * T
                        w_sb = wat.tile([128, 2, 2, T], F32R, tag="watt",
                                        name=f"w_{g}_{i}")
                        prsum = pvec.tile([1, 2 * T], F32, tag="vec",
                                          name=f"prs_{g}_{i}")
                        for ih in range(2):
                            prow = slice(ih * 64, ih * 64 + 64)
                            pwt = pat.tile([128, 2, T], F32, tag="at")
                            nc.tensor.matmul(
                                pwt[:, 0, :],
                                kT[prow, j, t0:t0 + 128],
                                qT[prow, j, t0:t0 + T],
                                start=True, stop=False, skip_group_check=True)
                            nc.tensor.matmul(
                                pwt[:, 0, 0:128], msk[:], idn[:],
                                start=False, stop=True, skip_group_check=True)
                            nc.tensor.matmul(
                                pwt[:, 1, :],
                                kT[prow, j, t0 + 128:t0 + T],
                                qT[prow, j, t0:t0 + T],
                                start=True, stop=False, skip_group_check=True)
                            nc.tensor.matmul(
                                pwt[:, 1, 0:128], mskf[:], idn[:],
                                start=False, stop=False, skip_group_check=True)
                            nc.tensor.matmul(
                                pwt[:, 1, 128:T], msk[:], idn[:],
                                start=False, stop=True, skip_group_check=True)
                            nc.scalar.activation(w_sb[:, ih, :, :],
                                                 pwt[:, :, :], ACT.Exp)
                            nc.tensor.matmul(
                                prsum[:, ih * T:ih * T + T], on128[:],
                                w_sb[:, ih, 0, :],
                                start=True, stop=False, skip_group_check=True)
                            nc.tensor.matmul(
                                prsum[:, ih * T:ih * T + T], on128[:],
                                w_sb[:, ih, 1, :],
                                start=False, stop=True, skip_group_check=True)
                        return w_sb, prsum

                    def stage_o(i, w_sb, prsum):
                        ib, j = iters[i]
                        t0 = ib * T
                        rf = sm.tile([1, 2 * T], F32, tag="rf")
                        nc.vector.reciprocal_approx_fast(out=rf[:], in_=prsum[:])
                        rr = sm.tile([1, 2 * T], F32R, tag="rr")
                        nc.vector.tensor_copy(out=rr[:], in_=rf[:])
                        prb = pat.tile([128, T], F32, tag="at")
                        nc.tensor.matmul(prb[:], sello[:], rr[:, 0:T],
                                         start=True, stop=False,
                                         skip_group_check=True)
                        nc.tensor.matmul(prb[:], selhi[:], rr[:, T:2 * T],
                                         start=False, stop=True,
                                         skip_group_check=True)
                        rb = sm.tile([128, T], F32, tag="rbs")
                        nc.scalar.activation(rb[:], prb[:], ACT.Copy)
                        psl = slice(j * 128, (j + 1) * 128)
                        for ih in range(2):
                            po = pat.tile([128, T], F32, tag="at")
                            nc.tensor.matmul(
                                po[:], v_sb[:, 2 * ib, psl], w_sb[:, ih, 0, :],
                                start=True, stop=False, skip_group_check=True)
                            nc.tensor.matmul(
                                po[:], v_sb[:, 2 * ib + 1, psl],
                                w_sb[:, ih, 1, :],
                                start=False, stop=True, skip_group_check=True)
                            hrow = slice(ih * 64, ih * 64 + 64)
                            nc.vector.tensor_mul(
                                out=oT[hrow, j, t0:t0 + T],
                                in0=po[hrow, :], in1=rb[hrow, :])

                    pend = stage_s(0)
                    for i in range(len(iters)):
                        cur = pend
                        if i + 1 < len(iters):
                            pend = stage_s(i + 1)
                        stage_o(i, *cur)
                    return oT

                def proj_phase(g, oT):
                    gsl = slice(g * TG, (g + 1) * TG)
                    for c in range(DC):
                        px = pmm.tile([128, TG], F32, tag="mm512")
                        dsl = slice(c * 128, (c + 1) * 128)
                        for kc in range(DC):
                            nc.tensor.matmul(px[:], wp_sb[:, kc, dsl], oT[:, kc, :],
                                             start=(kc == 0), stop=(kc == DC - 1))
                        if flags["bpr"]:
                            nc.vector.scalar_tensor_tensor(
                                out=xT[:, c, gsl], in0=px[:],
                                scalar=bpr_sb[:, l, c:c + 1], in1=xT[:, c, gsl],
                                op0=AluOpType.add, op1=AluOpType.add)
                        else:
                            nc.vector.tensor_add(
                                out=xT[:, c, gsl], in0=px[:], in1=xT[:, c, gsl])

                def ffn_phase(g, y2):
                    gsl = slice(g * TG, (g + 1) * TG)
                    h_sb = acts.tile([128, FC, TG], F32R, tag="h")
                    for mc in range(FC):
                        ph = pmm.tile([128, TG], F32, tag="mm512")
                        msl = slice(mc * 128, (mc + 1) * 128)
                        for kc in range(DC):
                            nc.tensor.matmul(ph[:], w1_sb[:, kc, msl], y2[:, kc, :],
                                             start=(kc == 0), stop=(kc == DC - 1))
                        if flags["b1"]:
                            nc.scalar.activation(h_sb[:, mc, :], ph[:], ACT.Relu,
                                                 bias=b1_sb[:, l, mc:mc + 1])
                        elif mc % 2 == 0:
                            nc.scalar.activation(h_sb[:, mc, :], ph[:], ACT.Relu)
                        else:
                            nc.vector.tensor_scalar_max(
                                out=h_sb[:, mc, :], in0=ph[:], scalar1=0.0)
                    px2s = [pmm.tile([128, TG], F32, tag="mm512", name=f"px2_{i}")
                            for i in range(DC)]
                    for kc in range(FC):
                        for c in range(DC):
                            dsl = slice(c * 128, (c + 1) * 128)
                            nc.tensor.matmul(px2s[c][:], w2_sb[:, kc, dsl],
                                             h_sb[:, kc, :],
                                             start=(kc == 0), stop=(kc == FC - 1),
                                             skip_group_check=True)
                    for c in range(DC):
                        px2 = px2s[c]
                        if flags["b2"]:
                            nc.vector.scalar_tensor_tensor(
                                out=xT[:, c, gsl], in0=px2[:],
                                scalar=b2_sb[:, l, c:c + 1], in1=xT[:, c, gsl],
                                op0=AluOpType.add, op1=AluOpType.add)
                        else:
                            nc.vector.tensor_add(
                                out=xT[:, c, gsl], in0=px2[:], in1=xT[:, c, gsl])

                # 1-group-lookahead pipeline: LN(g+1)'s DVE/ACT tail hides
                # under attn/proj/ffn(g) PE work
                def ln_tile(nm):
                    return lnp.tile([128, DC, TG], F32R, tag="ln_out", name=nm)

                if pending_yT0 is None:
                    yTs = {0: ln_tile(f"yT_{l}_0")}
                    layer_norm(xT[:, :, 0:TG], yTs[0], aff1)
                else:
                    yTs = {0: pending_yT0}
                for g in range(NG):
                    qkv = qkv_phase(g, yTs[g])
                    if g + 1 < NG:
                        yTs[g + 1] = ln_tile(f"yT_{l}_{g + 1}")
                        layer_norm(xT[:, :, (g + 1) * TG:(g + 2) * TG],
                                   yTs[g + 1], aff1)
                    oT = attn_phase(g, *qkv)
                    proj_phase(g, oT)
                y2s = {0: ln_tile(f"y2_{l}_0")}
                layer_norm(xT[:, :, 0:TG], y2s[0], aff1)
                for g in range(NG):
                    if g + 1 < NG:
                        y2s[g + 1] = ln_tile(f"y2_{l}_{g + 1}")
                        layer_norm(xT[:, :, (g + 1) * TG:(g + 2) * TG],
                                   y2s[g + 1], aff1)
                    ffn_phase(g, y2s[g])
                    if g == 2 and l + 1 < L:
                        if flags["ln1"]:
                            affn = (
                                lambda c, ln=l + 1: lg1_sb[:, ln, c:c + 1],
                                lambda c, ln=l + 1: lb1_sb[:, ln, c:c + 1],
                            )
                        else:
                            affn = None
                        pending_yT0 = ln_tile(f"yT_{l + 1}_0")
                        layer_norm(xT[:, :, 0:TG], pending_yT0, affn)

            # ---------- final LN + head + loss ----------
            if flags["lnf"]:
                afff = (lambda c: lgf_sb[:, c:c + 1], lambda c: lbf_sb[:, c:c + 1])
            else:
                afff = None
            for g in range(NG):
                gsl = slice(g * TG, (g + 1) * TG)
                yf = lnp.tile([128, DC, TG], F32R, tag="ln_out")
                layer_norm(xT[:, :, gsl], yf, afff)
                for j in range(4):
                    tile_idx = g * 4 + j
                    tsl = slice(j * 128, (j + 1) * 128)
                    pl = pmm.tile([128, TG], F32, tag="mm512")
                    for kc in range(DC):
                        nc.tensor.matmul(pl[:, :V + 1], yf[:, kc, tsl],
                                         whd[:, kc, :],
                                         start=(kc == 0), stop=(kc == DC - 1))
                    lg = io.tile([128, V], F32, tag="lg")
                    if flags["bhd"]:
                        nc.vector.tensor_add(out=lg[:], in0=pl[:, :V],
                                             in1=bhd_sb[:, :])
                    else:
                        nc.scalar.activation(lg[:], pl[:, :V], ACT.Copy)
                    nc.sync.dma_start(
                        out=LOGITS[tile_idx * 128:(tile_idx + 1) * 128, :],
                        in_=lg[:])
                    el = io.tile([128, V], F32, tag="el")
                    nc.scalar.activation(el[:], lg[:], ACT.Exp)
                    nc.vector.reduce_sum(ses[:, tile_idx:tile_idx + 1], el[:],
                                         axis=mybir.AxisListType.X)
                    ohtile = io.tile([128, V], BF16, tag="oht")
                    nc.sync.dma_start(
                        out=ohtile,
                        in_=OHT.rearrange("(n p) v -> p n v", p=128)[:, tile_idx, :])
                    dg = io.tile([128, V], F32, tag="dg")
                    nc.vector.tensor_mul(out=dg[:], in0=lg[:], in1=ohtile[:])
                    nc.vector.reduce_sum(dts[:, tile_idx:tile_idx + 1], dg[:],
                                         axis=mybir.AxisListType.X)
            lses = persist.tile([128, 16], F32, tag="lses")
            nc.scalar.activation(lses[:], ses[:], ACT.Ln)
            nc.vector.tensor_sub(out=lses[:], in0=lses[:], in1=dts[:])
            nc.vector.reduce_sum(acc[:, 0:1], lses[:], axis=mybir.AxisListType.X)
            accr = persist.tile([128, 2], F32R, tag="accr")
            nc.vector.tensor_copy(out=accr[:], in_=acc[:])
            plo = pvec.tile([1, 2], F32, tag="vec")
            nc.tensor.matmul(plo[:], on128[:], accr[:], start=True, stop=True)
            lo_sb = persist.tile([1, 1], F32, tag="lo")
            nc.scalar.activation(lo_sb[:], plo[0:1, 0:1], ACT.Copy)
            nc.sync.dma_start(out=LOSS[:, :], in_=lo_sb[:])

    nc.compile()
    return nc


def kernel(idx, targets, tok_emb, pos_emb, Wq, Wk, Wv, Wproj, bproj,
           W1, b1, W2, b2, ln1_g, ln1_b, lnf_g, lnf_b, Whead, bhead):
    idx = np.asarray(idx)
    targets = np.asarray(targets)
    f = np.asarray

    scale = float(D) ** -0.5
    flags = {
        "bpr": bool(np.any(f(bproj))),
        "b1": bool(np.any(f(b1))),
        "b2": bool(np.any(f(b2))),
        "ln1": bool(np.any(f(ln1_g) != 1.0) or np.any(f(ln1_b))),
        "lnf": bool(np.any(f(lnf_g) != 1.0) or np.any(f(lnf_b))),
        "bhd": bool(np.any(f(bhead))),
    }
    key = tuple(sorted(flags.items()))
    if key not in _cache:
        _cache[key] = _build_program(flags)
    nc = _cache[key]

    # ---- host-side shared arrays ----
    c32 = lambda a: np.ascontiguousarray(np.asarray(a, np.float32))
    emb_np = np.zeros((128, D), np.float32)
    emb_np[:V] = f(tok_emb)
    pos_np = c32(f(pos_emb).T.reshape(DC, 128, T))
    wq_np = c32(f(Wq).transpose(0, 2, 1, 3).reshape(L, D, D) * scale)
    wk_np = c32(f(Wk).transpose(0, 2, 1, 3).reshape(L, D, D))
    wv_np = c32(f(Wv).transpose(0, 2, 1, 3).reshape(L, D, D))
    wp_np = c32(Wproj)
    w1_np = c32(W1)
    w2_np = c32(W2)
    wh_np = np.zeros((D, V + 1), np.float32)
    wh_np[:, :V] = f(Whead)
    msk_np = np.zeros((128, 128), np.float32)
    s_i, t_i = np.meshgrid(np.arange(128), np.arange(128), indexing="ij")
    msk_np[t_i < s_i] = NEG   # mask[s, t] = NEG where t < s
    msk_np = msk_np.T.astype(ml_dtypes.bfloat16)  # lhsT = mask.T
    idn_np = np.eye(128, dtype=ml_dtypes.bfloat16)
    mskf_np = np.full((128, 128), NEG, ml_dtypes.bfloat16)
    on128_np = np.ones((128, 1), np.float32)
    onb_np = np.ones((128, 1), ml_dtypes.bfloat16)
    onk1_np = np.ones((1, 128), np.float32)
    sello_np = np.zeros((1, 128), np.float32)
    sello_np[0, :64] = 1.0
    selhi_np = np.zeros((1, 128), np.float32)
    selhi_np[0, 64:] = 1.0
    bpr_np = c32(f(bproj).reshape(L, DC, 128))
    b1_np = c32(f(b1).reshape(L, FC, 128))
    b2_np = c32(f(b2).reshape(L, DC, 128))
    lg1_np = c32(f(ln1_g).reshape(L, DC, 128))
    lb1_np = c32(f(ln1_b).reshape(L, DC, 128))
    lgf_np = c32(f(lnf_g).reshape(DC, 128))
    lbf_np = c32(f(lnf_b).reshape(DC, 128))
    bhd_np = np.broadcast_to(f(bhead).astype(np.float32), (128, V)).copy()

    shared = {
        "EMB": emb_np, "POS": pos_np, "WQ": wq_np, "WK": wk_np, "WV": wv_np,
        "WP": wp_np, "W1": w1_np, "W2": w2_np, "WH": wh_np, "MSK": msk_np,
        "IDN": idn_np, "MSKF": mskf_np, "ON128": on128_np, "ONB": onb_np, "ONK1": onk1_np, "SELLO": sello_np,
        "SELHI": selhi_np, "BPR": bpr_np, "B1": b1_np, "B2": b2_np,
        "LG1": lg1_np, "LB1": lb1_np, "LGF": lgf_np, "LBF": lbf_np,
        "BHD": bhd_np,
    }

    in_maps = []
    for c in range(NCORES):
        sl = slice(c * BL, (c + 1) * BL)
        idx_c = np.asarray(idx[sl]).reshape(NTOK)       # [2048]
        tgt_c = np.asarray(targets[sl]).reshape(NTOK)
        ioh = np.zeros((128, NTOK), np.float32)
        ioh[idx_c, np.arange(NTOK)] = 1.0
        oht = np.zeros((NTOK, V), ml_dtypes.bfloat16)
        oht[np.arange(NTOK), tgt_c] = 1.0
        in_maps.append({**shared, "IOH": ioh, "OHT": oht})

    res = bass_utils.run_bass_kernel_spmd(nc, in_maps, core_ids=list(range(NCORES)))

    logits = np.concatenate(
        [r["LOGITS"].reshape(BL, T, V) for r in res.results], axis=0)
    loss_sum = sum(float(r["LOSSN"][0, 0]) for r in res.results)
    loss = np.float32(loss_sum / (B * T))
    return logits, loss


# revision 45
# speedup vs baseline: 1.0416x; 1.0416x over previous
"""Trainium2 Bass kernel for a 6-layer bigram language model (dense transformer).

Data-parallel over batch: B=64 -> 8 sequences per NeuronCore, 8 cores.
Activations kept D-major (xT [D, tokens]); all matmuls in float32r
(full PE rate, ~1e-3 max rel err). Single ACT table set (exp/ln/relu/
square/copy). Returns (logits [64,256,65] f32, loss scalar f32).
"""
import ml_dtypes
import numpy as np

import concourse.bass as bass
import concourse.mybir as mybir
import concourse.tile as tile
from concourse import bacc, bass_utils
from concourse.alu_op_type import AluOpType

F32 = mybir.dt.float32
F32R = mybir.dt.float32r
BF16 = mybir.dt.bfloat16
ACT = mybir.ActivationFunctionType

L, H, HS, D, T, V, B = 6, 6, 64, 384, 256, 65, 64
EPS = 1e-5
NCORES = 8
BL = B // NCORES            # sequences per core = 8
NTOK = BL * T               # 2048 tokens per core
NG = 4                      # token groups per core
TG = NTOK // NG             # 512 tokens per group (2 sequences)
DC = D // 128               # 3 d-chunks
FF = 4 * D                  # 1536
FC = FF // 128              # 12 ffn chunks
NEG = -1.0e30

_cache = {}


def _build_program(flags):
    nc = bacc.Bacc("TRN2", target_bir_lowering=False, debug=False)

    # ---- DRAM tensors ----
    IOH = nc.dram_tensor("IOH", [128, NTOK], F32R, kind="ExternalInput")
    EMB = nc.dram_tensor("EMB", [128, D], F32R, kind="ExternalInput")
    POS = nc.dram_tensor("POS", [DC, 128, T], F32, kind="ExternalInput")
    WQ = nc.dram_tensor("WQ", [L, D, D], F32R, kind="ExternalInput")
    WK = nc.dram_tensor("WK", [L, D, D], F32R, kind="ExternalInput")
    WV = nc.dram_tensor("WV", [L, D, D], F32R, kind="ExternalInput")
    WP = nc.dram_tensor("WP", [L, D, D], F32R, kind="ExternalInput")
    W1 = nc.dram_tensor("W1", [L, D, FF], F32R, kind="ExternalInput")
    W2 = nc.dram_tensor("W2", [L, FF, D], F32R, kind="ExternalInput")
    WH = nc.dram_tensor("WH", [D, V + 1], F32R, kind="ExternalInput")
    MSK = nc.dram_tensor("MSK", [128, 128], BF16, kind="ExternalInput")
    MSKF = nc.dram_tensor("MSKF", [128, 128], BF16, kind="ExternalInput")
    IDN = nc.dram_tensor("IDN", [128, 128], BF16, kind="ExternalInput")
    ON128 = nc.dram_tensor("ON128", [128, 1], F32R, kind="ExternalInput")
    ONB = nc.dram_tensor("ONB", [128, 1], BF16, kind="ExternalInput")
    ONK1 = nc.dram_tensor("ONK1", [1, 128], F32R, kind="ExternalInput")
    SELLO = nc.dram_tensor("SELLO", [1, 128], F32R, kind="ExternalInput")
    SELHI = nc.dram_tensor("SELHI", [1, 128], F32R, kind="ExternalInput")
    OHT = nc.dram_tensor("OHT", [NTOK, V], BF16, kind="ExternalInput")
    # per-layer per-partition bias vectors (may be unused if all-zero)
    BPR = nc.dram_tensor("BPR", [L, DC, 128], F32, kind="ExternalInput")
    B1 = nc.dram_tensor("B1", [L, FC, 128], F32, kind="ExternalInput")
    B2 = nc.dram_tensor("B2", [L, DC, 128], F32, kind="ExternalInput")
    # LN affine (used only when nontrivial)
    LG1 = nc.dram_tensor("LG1", [L, DC, 128], F32, kind="ExternalInput")
    LB1 = nc.dram_tensor("LB1", [L, DC, 128], F32, kind="ExternalInput")
    LGF = nc.dram_tensor("LGF", [DC, 128], F32, kind="ExternalInput")
    LBF = nc.dram_tensor("LBF", [DC, 128], F32, kind="ExternalInput")
    BHD = nc.dram_tensor("BHD", [128, V], F32, kind="ExternalInput")

    LOGITS = nc.dram_tensor("LOGITS", [NTOK, V], F32, kind="ExternalOutput")
    LOSS = nc.dram_tensor("LOSSN", [1, 1], F32, kind="ExternalOutput")

    with tile.TileContext(nc) as tc:
        with (
            tc.tile_pool(name="persist", bufs=1) as persist,
            tc.tile_pool(name="wts", bufs=1) as wts,
            tc.tile_pool(name="wts1", bufs=1) as wts1,
            tc.tile_pool(name="acts", bufs=1) as acts,
            tc.tile_pool(name="lnp", bufs=2) as lnp,
            tc.tile_pool(name="sm", bufs=2) as sm,
            tc.tile_pool(name="wat", bufs=3) as wat,
            tc.tile_pool(name="io", bufs=2) as io,
            tc.tile_pool(name="pmm", bufs=3, space="PSUM") as pmm,
            tc.tile_pool(name="pat", bufs=3, space="PSUM") as pat,
            tc.tile_pool(name="pvec", bufs=2, space="PSUM") as pvec,
        ):
            # ---------- constants ----------
            xT = persist.tile([128, DC, NTOK], F32R, tag="xT")
            emb = persist.tile([128, D], F32R, tag="emb")
            nc.sync.dma_start(out=emb, in_=EMB[:, :])
            pos = persist.tile([128, DC, T], F32, tag="pos")
            nc.sync.dma_start(out=pos, in_=POS[:, :, :].rearrange("c p t -> p c t"))
            msk = persist.tile([128, 128], BF16, tag="msk")
            nc.sync.dma_start(out=msk, in_=MSK[:, :])
            mskf = persist.tile([128, 128], BF16, tag="mskf")
            nc.sync.dma_start(out=mskf, in_=MSKF[:, :])
            idn = persist.tile([128, 128], BF16, tag="idn")
            nc.sync.dma_start(out=idn, in_=IDN[:, :])
            on128 = persist.tile([128, 1], F32R, tag="on128")
            nc.sync.dma_start(out=on128, in_=ON128[:, :])
            onb = persist.tile([128, 1], BF16, tag="onb")
            nc.sync.dma_start(out=onb, in_=ONB[:, :])
            onk1 = persist.tile([1, 128], F32R, tag="onk1")
            nc.sync.dma_start(out=onk1, in_=ONK1[:, :])
            sello = persist.tile([1, 128], F32R, tag="sello")
            nc.sync.dma_start(out=sello, in_=SELLO[:, :])
            selhi = persist.tile([1, 128], F32R, tag="selhi")
            nc.sync.dma_start(out=selhi, in_=SELHI[:, :])
            whd = persist.tile([128, DC, V + 1], F32R, tag="whd")
            nc.sync.dma_start(out=whd, in_=WH.rearrange("(c p) v -> p c v", p=128))

            acc = persist.tile([128, 2], F32, tag="acc")
            nc.vector.memset(acc, 0.0)
            ses = persist.tile([128, 16], F32, tag="ses")
            dts = persist.tile([128, 16], F32, tag="dts")
            eps_sb = persist.tile([1, 1], F32, tag="eps")
            nc.vector.memset(eps_sb, EPS)

            bpr_sb = persist.tile([128, L, DC], F32, tag="bpr")
            b2_sb = persist.tile([128, L, DC], F32, tag="b2")
            b1_sb = persist.tile([128, L, FC], F32, tag="b1")
            if flags["bpr"]:
                nc.sync.dma_start(out=bpr_sb, in_=BPR.rearrange("l c p -> p l c"))
            if flags["b2"]:
                nc.sync.dma_start(out=b2_sb, in_=B2.rearrange("l c p -> p l c"))
            if flags["b1"]:
                nc.sync.dma_start(out=b1_sb, in_=B1.rearrange("l c p -> p l c"))
            lg1_sb = persist.tile([128, L, DC], F32, tag="lg1")
            lb1_sb = persist.tile([128, L, DC], F32, tag="lb1")
            lgf_sb = persist.tile([128, DC], F32, tag="lgf")
            lbf_sb = persist.tile([128, DC], F32, tag="lbf")
            if flags["ln1"]:
                nc.sync.dma_start(out=lg1_sb, in_=LG1.rearrange("l c p -> p l c"))
                nc.sync.dma_start(out=lb1_sb, in_=LB1.rearrange("l c p -> p l c"))
            if flags["lnf"]:
                nc.sync.dma_start(out=lgf_sb, in_=LGF.rearrange("c p -> p c"))
                nc.sync.dma_start(out=lbf_sb, in_=LBF.rearrange("c p -> p c"))
            bhd_sb = persist.tile([128, V], F32, tag="bhd")
            if flags["bhd"]:
                nc.sync.dma_start(out=bhd_sb, in_=BHD[:, :])

            # ---------- embedding ----------
            for g in range(NG):
                gsl = slice(g * TG, (g + 1) * TG)
                i_sb = io.tile([128, TG], F32R, tag="ioh")
                nc.sync.dma_start(out=i_sb, in_=IOH[:, gsl])
                for c in range(DC):
                    pe = pmm.tile([128, TG], F32, tag="mm512")
                    nc.tensor.matmul(
                        pe[:], emb[:, c * 128:(c + 1) * 128], i_sb[:],
                        start=True, stop=True,
                    )
                    for half in range(2):
                        hsl = slice(half * T, (half + 1) * T)
                        nc.vector.tensor_add(
                            out=xT[:, c, g * TG + half * T: g * TG + (half + 1) * T],
                            in0=pe[:, hsl], in1=pos[:, c, :],
                        )

            # ---------- layer norm helper ----------
            def layer_norm(src_view, dst, affine):
                """src_view: [128, DC, TG] f32r AP; dst [128, DC, TG] f32r tile."""
                sq = lnp.tile([128, DC, TG], F32R, tag="sq")
                for c in range(DC):
                    nc.scalar.activation(sq[:, c, :], src_view[:, c, :], ACT.Square)
                ps1 = pvec.tile([1, TG], F32, tag="vec")
                ps2 = pvec.tile([1, TG], F32, tag="vec")
                for c in range(DC):
                    nc.tensor.matmul(ps1[:], on128[:], src_view[:, c, :],
                                     start=(c == 0), stop=(c == DC - 1))
                    nc.tensor.matmul(ps2[:], on128[:], sq[:, c, :],
                                     start=(c == 0), stop=(c == DC - 1))
                m = sm.tile([1, TG], F32, tag="stat_m")
                nc.scalar.activation(m[:], ps1[:], ACT.Copy, scale=1.0 / D)
                mm = sm.tile([1, TG], F32, tag="stat")
                nc.vector.tensor_mul(out=mm[:], in0=m[:], in1=m[:])
                var = sm.tile([1, TG], F32, tag="stat")
                nc.vector.scalar_tensor_tensor(
                    out=var[:], in0=ps2[:], scalar=1.0 / D, in1=mm[:],
                    op0=AluOpType.mult, op1=AluOpType.subtract,
                )
                lnv = sm.tile([1, TG], F32, tag="stat")
                nc.scalar.activation(lnv[:], var[:], ACT.Ln, bias=eps_sb[:])
                a_r = sm.tile([1, TG], F32R, tag="statr")
                nc.scalar.activation(a_r[:], lnv[:], ACT.Exp, scale=-0.5)
                ma_r = sm.tile([1, TG], F32R, tag="statr")
                nc.vector.tensor_mul(out=ma_r[:], in0=m[:], in1=a_r[:])
                pa = pat.tile([128, TG], F32, tag="at")
                nc.tensor.matmul(pa[:], onk1[:], a_r[:], start=True, stop=True)
                pma = pat.tile([128, TG], F32, tag="at")
                nc.tensor.matmul(pma[:], onk1[:], ma_r[:], start=True, stop=True)
                for c in range(DC):
                    u = io.tile([128, TG], F32, tag="u")
                    nc.vector.tensor_mul(out=u[:], in0=src_view[:, c, :], in1=pa[:])
                    if affine is None:
                        nc.vector.tensor_sub(out=dst[:, c, :], in0=u[:], in1=pma[:])
                    else:
                        nc.vector.tensor_sub(out=u[:], in0=u[:], in1=pma[:])
                        gs, bs = affine
                        nc.vector.tensor_scalar(
                            out=dst[:, c, :], in0=u[:],
                            scalar1=gs(c), scalar2=bs(c),
                            op0=AluOpType.mult, op1=AluOpType.add,
                        )

            # ---------- transformer layers ----------
            pending_yT0 = None
            for l in range(L):
                wq_sb = wts.tile([128, DC, D], F32R, tag="wq")
                nc.sync.dma_start(out=wq_sb, in_=WQ[l].rearrange("(c p) e -> p c e", p=128))
                wk_sb = wts.tile([128, DC, D], F32R, tag="wk")
                nc.sync.dma_start(out=wk_sb, in_=WK[l].rearrange("(c p) e -> p c e", p=128))
                wv_sb = wts.tile([128, DC, D], F32R, tag="wv")
                nc.sync.dma_start(out=wv_sb, in_=WV[l].rearrange("(c p) e -> p c e", p=128))
                wp_sb = wts.tile([128, DC, D], F32R, tag="wp")
                nc.sync.dma_start(out=wp_sb, in_=WP[l].rearrange("(c p) e -> p c e", p=128))
                w1_sb = wts1.tile([128, DC, FF], F32R, tag="w1")
                nc.sync.dma_start(out=w1_sb, in_=W1[l].rearrange("(c p) e -> p c e", p=128))
                w2_sb = wts1.tile([128, FC, D], F32R, tag="w2")
                nc.sync.dma_start(out=w2_sb, in_=W2[l].rearrange("(c p) e -> p c e", p=128))

                if flags["ln1"]:
                    aff1 = (
                        lambda c, l=l: lg1_sb[:, l, c:c + 1],
                        lambda c, l=l: lb1_sb[:, l, c:c + 1],
                    )
                else:
                    aff1 = None

                def qkv_phase(g, yT):
                    gsl = slice(g * TG, (g + 1) * TG)
                    qT = acts.tile([128, DC, TG], F32R, tag="qT", name=f"qT{g}")
                    kT = acts.tile([128, DC, TG], F32R, tag="kT", name=f"kT{g}")
                    for ec in range(DC):
                        pq = pmm.tile([128, TG], F32, tag="mm512")
                        pk = pmm.tile([128, TG], F32, tag="mm512")
                        for kc in range(DC):
                            esl = slice(ec * 128, (ec + 1) * 128)
                            nc.tensor.matmul(pq[:], wq_sb[:, kc, esl], yT[:, kc, :],
                                             start=(kc == 0), stop=(kc == DC - 1))
                            nc.tensor.matmul(pk[:], wk_sb[:, kc, esl], yT[:, kc, :],
                                             start=(kc == 0), stop=(kc == DC - 1))
                        nc.vector.tensor_copy(out=qT[:, ec, :], in_=pq[:])
                        nc.scalar.activation(kT[:, ec, :], pk[:], ACT.Copy)
                    v_sb = acts.tile([128, 4, D], F32R, tag="v", name=f"v{g}")
                    for j in range(4):
                        pv = pmm.tile([128, TG], F32, tag="mm512")
                        tsl = slice(j * 128, (j + 1) * 128)
                        for kc in range(DC):
                            nc.tensor.matmul(pv[:, :D], yT[:, kc, tsl], wv_sb[:, kc, :],
                                             start=(kc == 0), stop=(kc == DC - 1))
                        nc.scalar.activation(v_sb[:, j, :], pv[:, :D], ACT.Copy)
                    return qT, kT, v_sb

                def attn_phase(g, qT, kT, v_sb):
                    oT = acts.tile([128, DC, TG], F32R, tag="oT", name=f"oT{g}")
                    iters = [(ib, j) for ib in range(2) for j in range(DC)]

                    def stage_s(i):
                        ib, j = iters[i]
                        t0 = ib * T
                        w_sb = wat.tile([128, 2, 2, T], F32R, tag="watt",
                                        name=f"w_{g}_{i}")
                        prsum = pvec.tile([1, 2 * T], F32, tag="vec",
                                          name=f"prs_{g}_{i}")
                        for ih in range(2):
                            prow = slice(ih * 64, ih * 64 + 64)
                            pwt = pat.tile([128, 2, T], F32, tag="at")
                            nc.tensor.matmul(
                                pwt[:, 0, :],
                                kT[prow, j, t0:t0 + 128],
                                qT[prow, j, t0:t0 + T],
                                start=True, stop=False, skip_group_check=True)
                            nc.tensor.matmul(
                                pwt[:, 0, 0:128], msk[:], idn[:],
                                start=False, stop=True, skip_group_check=True)
                            nc.tensor.matmul(
                                pwt[:, 1, :],
                                kT[prow, j, t0 + 128:t0 + T],
                                qT[prow, j, t0:t0 + T],
                                start=True, stop=False, skip_group_check=True)
                            nc.tensor.matmul(
                                pwt[:, 1, 0:128], mskf[:], idn[:],
                                start=False, stop=False, skip_group_check=True)
                            nc.tensor.matmul(
                                pwt[:, 1, 128:T], msk[:], idn[:],
                                start=False, stop=True, skip_group_check=True)
                            nc.scalar.activation(w_sb[:, ih, :, :],
                                                 pwt[:, :, :], ACT.Exp)
                            nc.tensor.matmul(
                                prsum[:, ih * T:ih * T + T], on128[:],
                                w_sb[:, ih, 0, :],
                                start=True, stop=False, skip_group_check=True)
                            nc.tensor.matmul(
                                prsum[:, ih * T:ih * T + T], on128[:],
                                w_sb[:, ih, 1, :],
                                start=False, stop=True, skip_group_check=True)
                        return w_sb, prsum

                    def stage_o(i, w_sb, prsum):
                        ib, j = iters[i]
                        t0 = ib * T
                        rf = sm.tile([1, 2 * T], F32, tag="rf")
                        nc.vector.reciprocal_approx_fast(out=rf[:], in_=prsum[:])
                        rr = sm.tile([1, 2 * T], F32R, tag="rr")
                        nc.vector.tensor_copy(out=rr[:], in_=rf[:])
                        prb = pat.tile([128, T], F32, tag="at")
                        nc.tensor.matmul(prb[:], sello[:], rr[:, 0:T],
                                         start=True, stop=False,
                                         skip_group_check=True)
                        nc.tensor.matmul(prb[:], selhi[:], rr[:, T:2 * T],
                                         start=False, stop=True,
                                         skip_group_check=True)
                        rb = sm.tile([128, T], F32, tag="rbs")
                        nc.vector.tensor_copy(out=rb[:], in_=prb[:])
                        psl = slice(j * 128, (j + 1) * 128)
                        for ih in range(2):
                            po = pat.tile([128, T], F32, tag="at")
                            nc.tensor.matmul(
                                po[:], v_sb[:, 2 * ib, psl], w_sb[:, ih, 0, :],
                                start=True, stop=False, skip_group_check=True)
                            nc.tensor.matmul(
                                po[:], v_sb[:, 2 * ib + 1, psl],
                                w_sb[:, ih, 1, :],
                                start=False, stop=True, skip_group_check=True)
                            hrow = slice(ih * 64, ih * 64 + 64)
                            nc.vector.tensor_mul(
                                out=oT[hrow, j, t0:t0 + T],
                                in0=po[hrow, :], in1=rb[hrow, :])

                    pend = stage_s(0)
                    for i in range(len(iters)):
                        cur = pend
                        if i + 1 < len(iters):
                            pend = stage_s(i + 1)
                        stage_o(i, *cur)
                    return oT

                def proj_phase(g, oT):
                    gsl = slice(g * TG, (g + 1) * TG)
                    for c in range(DC):
                        px = pmm.tile([128, TG], F32, tag="mm512")
                        dsl = slice(c * 128, (c + 1) * 128)
                        for kc in range(DC):
                            nc.tensor.matmul(px[:], wp_sb[:, kc, dsl], oT[:, kc, :],
                                             start=(kc == 0), stop=(kc == DC - 1))
                        if flags["bpr"]:
                            nc.vector.scalar_tensor_tensor(
                                out=xT[:, c, gsl], in0=px[:],
                                scalar=bpr_sb[:, l, c:c + 1], in1=xT[:, c, gsl],
                                op0=AluOpType.add, op1=AluOpType.add)
                        else:
                            nc.vector.tensor_add(
                                out=xT[:, c, gsl], in0=px[:], in1=xT[:, c, gsl])

                def ffn_phase(g, y2):
                    gsl = slice(g * TG, (g + 1) * TG)
                    h_sb = acts.tile([128, FC, TG], F32R, tag="h")
                    for mc in range(FC):
                        ph = pmm.tile([128, TG], F32, tag="mm512")
                        msl = slice(mc * 128, (mc + 1) * 128)
                        for kc in range(DC):
                            nc.tensor.matmul(ph[:], w1_sb[:, kc, msl], y2[:, kc, :],
                                             start=(kc == 0), stop=(kc == DC - 1))
                        if flags["b1"]:
                            nc.scalar.activation(h_sb[:, mc, :], ph[:], ACT.Relu,
                                                 bias=b1_sb[:, l, mc:mc + 1])
                        elif mc % 2 == 0:
                            nc.scalar.activation(h_sb[:, mc, :], ph[:], ACT.Relu)
                        else:
                            nc.vector.tensor_scalar_max(
                                out=h_sb[:, mc, :], in0=ph[:], scalar1=0.0)
                    px2s = [pmm.tile([128, TG], F32, tag="mm512", name=f"px2_{i}")
                            for i in range(DC)]
                    for kc in range(FC):
                        for c in range(DC):
                            dsl = slice(c * 128, (c + 1) * 128)
                            nc.tensor.matmul(px2s[c][:], w2_sb[:, kc, dsl],
                                             h_sb[:, kc, :],
                                             start=(kc == 0), stop=(kc == FC - 1),
                                             skip_group_check=True)
                    for c in range(DC):
                        px2 = px2s[c]
                        if flags["b2"]:
                            nc.vector.scalar_tensor_tensor(
                                out=xT[:, c, gsl], in0=px2[:],
                                scalar=b2_sb[:, l, c:c + 1], in1=xT[:, c, gsl],
                                op0=AluOpType.add, op1=AluOpType.add)
                        else:
                            nc.vector.tensor_add(
                                out=xT[:, c, gsl], in0=px2[:], in1=xT[:, c, gsl])

                # 1-group-lookahead pipeline: LN(g+1)'s DVE/ACT tail hides
                # under attn/proj/ffn(g) PE work
                def ln_tile(nm):
                    return lnp.tile([128, DC, TG], F32R, tag="ln_out", name=nm)

                if pending_yT0 is None:
                    yTs = {0: ln_tile(f"yT_{l}_0")}
                    layer_norm(xT[:, :, 0:TG], yTs[0], aff1)
                else:
                    yTs = {0: pending_yT0}
                for g in range(NG):
                    qkv = qkv_phase(g, yTs[g])
                    if g + 1 < NG:
                        yTs[g + 1] = ln_tile(f"yT_{l}_{g + 1}")
                        layer_norm(xT[:, :, (g + 1) * TG:(g + 2) * TG],
                                   yTs[g + 1], aff1)
                    oT = attn_phase(g, *qkv)
                    proj_phase(g, oT)
                y2s = {0: ln_tile(f"y2_{l}_0")}
                layer_norm(xT[:, :, 0:TG], y2s[0], aff1)
                for g in range(NG):
                    if g + 1 < NG:
                        y2s[g + 1] = ln_tile(f"y2_{l}_{g + 1}")
                        layer_norm(xT[:, :, (g + 1) * TG:(g + 2) * TG],
                                   y2s[g + 1], aff1)
                    ffn_phase(g, y2s[g])
                    if g == 2 and l + 1 < L:
                        if flags["ln1"]:
                            affn = (
                                lambda c, ln=l + 1: lg1_sb[:, ln, c:c + 1],
                                lambda c, ln=l + 1: lb1_sb[:, ln, c:c + 1],
                            )
                        else:
                            affn = None
                        pending_yT0 = ln_tile(f"yT_{l + 1}_0")
                        layer_norm(xT[:, :, 0:TG], pending_yT0, affn)

            # ---------- final LN + head + loss ----------
            if flags["lnf"]:
                afff = (lambda c: lgf_sb[:, c:c + 1], lambda c: lbf_sb[:, c:c + 1])
            else:
                afff = None
            for g in range(NG):
                gsl = slice(g * TG, (g + 1) * TG)
                yf = lnp.tile([128, DC, TG], F32R, tag="ln_out")
                layer_norm(xT[:, :, gsl], yf, afff)
                for j in range(4):
                    tile_idx = g * 4 + j
                    tsl = slice(j * 128, (j + 1) * 128)
                    pl = pmm.tile([128, TG], F32, tag="mm512")
                    for kc in range(DC):
                        nc.tensor.matmul(pl[:, :V + 1], yf[:, kc, tsl],
                                         whd[:, kc, :],
                                         start=(kc == 0), stop=(kc == DC - 1))
                    lg = io.tile([128, V], F32, tag="lg")
                    if flags["bhd"]:
                        nc.vector.tensor_add(out=lg[:], in0=pl[:, :V],
                                             in1=bhd_sb[:, :])
                    else:
                        nc.scalar.activation(lg[:], pl[:, :V], ACT.Copy)
                    nc.sync.dma_start(
                        out=LOGITS[tile_idx * 128:(tile_idx + 1) * 128, :],
                        in_=lg[:])
                    el = io.tile([128, V], F32, tag="el")
                    nc.scalar.activation(el[:], lg[:], ACT.Exp)
                    nc.vector.reduce_sum(ses[:, tile_idx:tile_idx + 1], el[:],
                                         axis=mybir.AxisListType.X)
                    ohtile = io.tile([128, V], BF16, tag="oht")
                    nc.sync.dma_start(
                        out=ohtile,
                        in_=OHT.rearrange("(n p) v -> p n v", p=128)[:, tile_idx, :])
                    dg = io.tile([128, V], F32, tag="dg")
                    nc.vector.tensor_mul(out=dg[:], in0=lg[:], in1=ohtile[:])
                    nc.vector.reduce_sum(dts[:, tile_idx:tile_idx + 1], dg[:],
                                         axis=mybir.AxisListType.X)
            lses = persist.tile([128, 16], F32, tag="lses")
            nc.scalar.activation(lses[:], ses[:], ACT.Ln)
            nc.vector.tensor_sub(out=lses[:], in0=lses[:], in1=dts[:])
            nc.vector.reduce_sum(acc[:, 0:1], lses[:], axis=mybir.AxisListType.X)
            accr = persist.tile([128, 2], F32R, tag="accr")
            nc.vector.tensor_copy(out=accr[:], in_=acc[:])
            plo = pvec.tile([1, 2], F32, tag="vec")
            nc.tensor.matmul(plo[:], on128[:], accr[:], start=True, stop=True)
            lo_sb = persist.tile([1, 1], F32, tag="lo")
            nc.scalar.activation(lo_sb[:], plo[0:1, 0:1], ACT.Copy)
            nc.sync.dma_start(out=LOSS[:, :], in_=lo_sb[:])

    nc.compile()
    return nc


def kernel(idx, targets, tok_emb, pos_emb, Wq, Wk, Wv, Wproj, bproj,
           W1, b1, W2, b2, ln1_g, ln1_b, lnf_g, lnf_b, Whead, bhead):
    idx = np.asarray(idx)
    targets = np.asarray(targets)
    f = np.asarray

    scale = float(D) ** -0.5
    flags = {
        "bpr": bool(np.any(f(bproj))),
        "b1": bool(np.any(f(b1))),
        "b2": bool(np.any(f(b2))),
        "ln1": bool(np.any(f(ln1_g) != 1.0) or np.any(f(ln1_b))),
        "lnf": bool(np.any(f(lnf_g) != 1.0) or np.any(f(lnf_b))),
        "bhd": bool(np.any(f(bhead))),
    }
    key = tuple(sorted(flags.items()))
    if key not in _cache:
        _cache[key] = _build_program(flags)
    nc = _cache[key]

    # ---- host-side shared arrays ----
    c32 = lambda a: np.ascontiguousarray(np.asarray(a, np.float32))
    emb_np = np.zeros((128, D), np.float32)
    emb_np[:V] = f(tok_emb)
    pos_np = c32(f(pos_emb).T.reshape(DC, 128, T))
    wq_np = c32(f(Wq).transpose(0, 2, 1, 3).reshape(L, D, D) * scale)
    wk_np = c32(f(Wk).transpose(0, 2, 1, 3).reshape(L, D, D))
    wv_np = c32(f(Wv).transpose(0, 2, 1, 3).reshape(L, D, D))
    wp_np = c32(Wproj)
    w1_np = c32(W1)
    w2_np = c32(W2)
    wh_np = np.zeros((D, V + 1), np.float32)
    wh_np[:, :V] = f(Whead)
    msk_np = np.zeros((128, 128), np.float32)
    s_i, t_i = np.meshgrid(np.arange(128), np.arange(128), indexing="ij")
    msk_np[t_i < s_i] = NEG   # mask[s, t] = NEG where t < s
    msk_np = msk_np.T.astype(ml_dtypes.bfloat16)  # lhsT = mask.T
    idn_np = np.eye(128, dtype=ml_dtypes.bfloat16)
    mskf_np = np.full((128, 128), NEG, ml_dtypes.bfloat16)
    on128_np = np.ones((128, 1), np.float32)
    onb_np = np.ones((128, 1), ml_dtypes.bfloat16)
    onk1_np = np.ones((1, 128), np.float32)
    sello_np = np.zeros((1, 128), np.float32)
    sello_np[0, :64] = 1.0
    selhi_np = np.zeros((1, 128), np.float32)
    selhi_np[0, 64:] = 1.0
    bpr_np = c32(f(bproj).reshape(L, DC, 128))
    b1_np = c32(f(b1).reshape(L, FC, 128))
    b2_np = c32(f(b2).reshape(L, DC, 128))
    lg1_np = c32(f(ln1_g).reshape(L, DC, 128))
    lb1_np = c32(f(ln1_b).reshape(L, DC, 128))
    lgf_np = c32(f(lnf_g).reshape(DC, 128))
    lbf_np = c32(f(lnf_b).reshape(DC, 128))
    bhd_np = np.broadcast_to(f(bhead).astype(np.float32), (128, V)).copy()

    shared = {
        "EMB": emb_np, "POS": pos_np, "WQ": wq_np, "WK": wk_np, "WV": wv_np,
        "WP": wp_np, "W1": w1_np, "W2": w2_np, "WH": wh_np, "MSK": msk_np,
        "IDN": idn_np, "MSKF": mskf_np, "ON128": on128_np, "ONB": onb_np, "ONK1": onk1_np, "SELLO": sello_np,
        "SELHI": selhi_np, "BPR": bpr_np, "B1": b1_np, "B2": b2_np,
        "LG1": lg1_np, "LB1": lb1_np, "LGF": lgf_np, "LBF": lbf_np,
        "BHD": bhd_np,
    }

    in_maps = []
    for c in range(NCORES):
        sl = slice(c * BL, (c + 1) * BL)
        idx_c = np.asarray(idx[sl]).reshape(NTOK)       # [2048]
        tgt_c = np.asarray(targets[sl]).reshape(NTOK)
        ioh = np.zeros((128, NTOK), np.float32)
        ioh[idx_c, np.arange(NTOK)] = 1.0
        oht = np.zeros((NTOK, V), ml_dtypes.bfloat16)
        oht[np.arange(NTOK), tgt_c] = 1.0
        in_maps.append({**shared, "IOH": ioh, "OHT": oht})

    res = bass_utils.run_bass_kernel_spmd(nc, in_maps, core_ids=list(range(NCORES)))

    logits = np.concatenate(
        [r["LOGITS"].reshape(BL, T, V) for r in res.results], axis=0)
    loss_sum = sum(float(r["LOSSN"][0, 0]) for r in res.results)
    loss = np.float32(loss_sum / (B * T))
    return logits, loss
